# revision 4
# baseline (speedup 1.0000x reference)
"""Distributed single-head attention for Trainium2 (8 NeuronCores).

Problem: out = softmax((Q Wq + bq)(K Wk + bk)^T / sqrt(64)) (V Wv + bv)
with Q,K,V [4, 4096, 1024], Wq/Wk/Wv [1024, 64].

Sharding: core c = 2*b + h handles batch b, query-sequence half h.
Q/K/V shards are pre-transposed on the host to [1024, 2048] so the
d_model contraction lands on SBUF partitions with no on-device
transposes.  k^T and v^T are computed from the local K/V seq-half and
pair-AllGathered (cores {2b, 2b+1}), then each core runs full attention
for its 2048 query rows against all 4096 keys.

Per-core pipeline:
  qT/kT/vT = W^T @ X^T + b        (fp32r matmuls, PSUM accumulate)
  AllGather [kT;vT] over the batch pair
  v1 = [v | 1] natural-layout bf16 (PE transpose of vT)
  S^T tiles = kT_tile^T @ qT       (fp32r), P^T = exp(S^T/8) in bf16
  out_unnorm[s, 0:65] = sum_t P^T_chunk^T @ v1_chunk  (bf16 matmuls)
  out = out_unnorm[:, :64] / out_unnorm[:, 64]
"""

import sys

if "/opt/trn_rl_repo" not in sys.path:
    sys.path.insert(0, "/opt/trn_rl_repo")

from contextlib import ExitStack

import numpy as np

import concourse.bass as bass
import concourse.tile as tile
from concourse import bacc, mybir
from concourse.bass_utils import run_bass_kernel_spmd

F32 = mybir.dt.float32
F32R = mybir.dt.float32r
BF16 = mybir.dt.bfloat16

B, S, DM, DK = 4, 4096, 1024, 64
N_CORES = 8
SH = S // 2          # per-core query/key rows (2048)
NCH = DM // 128      # d_model chunks (8)
SB = 512             # s-block size for the attention pipeline
NSB = SH // SB       # 4 s-blocks
NTT = SH // 128      # t-tiles per rank half (16)

_compiled = None


def _projection(nc, ctx, tc, xt_pool, psum_proj, src_dram, w_sb, w_col, b_sb,
                dst_ap_fn):
    """dst[0:64, :] = W^T @ X^T + b for one input tensor.

    w_sb: SBUF tile [128, 3*8*64] holding all weight chunks; w_col = base
    column of this tensor's 8 chunks. dst_ap_fn(st) -> [64, 512] AP.
    """
    psums = []
    for st in range(NSB):
        psums.append(psum_proj.tile([DK, SB], F32, tag="proj", name=f"proj_ps{st}"))
    for ch in range(NCH):
        xt = xt_pool.tile([128, SH], F32R, tag="xin")
        nc.sync.dma_start(xt[:, :], src_dram.ap().bitcast(F32R)[ch * 128:(ch + 1) * 128, :])
        for st in range(NSB):
            nc.tensor.matmul(
                psums[st][:, :],
                lhsT=w_sb[:, w_col + ch * DK: w_col + (ch + 1) * DK],
                rhs=xt[:, st * SB:(st + 1) * SB],
                start=(ch == 0),
                stop=(ch == NCH - 1),
            )
    for st in range(NSB):
        nc.scalar.activation(
            dst_ap_fn(st), psums[st][:, :],
            mybir.ActivationFunctionType.Identity, bias=b_sb[:, :],
        )


def _build():
    nc = bacc.Bacc("TRN2", target_bir_lowering=False, debug=False,
                   num_devices=N_CORES)

    qt_d = nc.dram_tensor("QT", [DM, SH], F32, kind="ExternalInput")
    kt_d = nc.dram_tensor("KT", [DM, SH], F32, kind="ExternalInput")
    vt_d = nc.dram_tensor("VT", [DM, SH], F32, kind="ExternalInput")
    w_d = {}
    b_d = {}
    for nm in ("q", "k", "v"):
        w_d[nm] = nc.dram_tensor(f"W{nm}", [DM, DK], F32, kind="ExternalInput")
        b_d[nm] = nc.dram_tensor(f"b{nm}", [DK, 1], F32, kind="ExternalInput")
    out_d = nc.dram_tensor("out", [SH, DK], F32, kind="ExternalOutput")

    ident_d = nc.inline_tensor(np.eye(128, dtype=np.float32), name="ident")

    with tile.TileContext(nc) as tc, ExitStack() as ctx:
        const = ctx.enter_context(tc.tile_pool(name="const", bufs=1))
        dram = ctx.enter_context(tc.tile_pool(name="dram", bufs=1, space="DRAM"))
        persist = ctx.enter_context(tc.tile_pool(name="persist", bufs=1))
        xt_pool = ctx.enter_context(tc.tile_pool(name="xin", bufs=3))

        # --- constants ---------------------------------------------------
        ident = const.tile([128, 128], F32R, tag="ident")
        nc.sync.dma_start(ident[:, :], ident_d.ap().bitcast(F32R)[:, :])
        w_sb = const.tile([128, 3 * NCH * DK], F32R, tag="w")
        b_sb = {}
        for i, nm in enumerate(("q", "k", "v")):
            nc.sync.dma_start(
                w_sb[:, :].rearrange("p (w c n) -> p w c n", w=3, n=DK)[:, i],
                w_d[nm].ap().bitcast(F32R).rearrange("(c p) n -> p c n", p=128),
            )
            b_sb[nm] = const.tile([DK, 1], F32, tag=f"b{nm}", name=f"b{nm}_sb")
            nc.sync.dma_start(b_sb[nm][:, :], b_d[nm].ap()[:, :])

        # --- projections -------------------------------------------------
        # kvT_sb rows 0:64 = kT_local, rows 64:128 = vT_local
        kvt_sb = persist.tile([128, SH], F32, tag="kvt")
        qt_sb = persist.tile([DK, SH], F32R, tag="qt")

        with tc.tile_pool(name="psum_proj", bufs=4, space="PSUM") as psum_proj:
            _projection(nc, ctx, tc, xt_pool, psum_proj, kt_d, w_sb, NCH * DK,
                        b_sb["k"], lambda st: kvt_sb[0:DK, st * SB:(st + 1) * SB])
            _projection(nc, ctx, tc, xt_pool, psum_proj, vt_d, w_sb, 2 * NCH * DK,
                        b_sb["v"], lambda st: kvt_sb[DK:128, st * SB:(st + 1) * SB])

            # --- pair AllGather of [kT; vT] ------------------------------
            ag_in = dram.tile([128, SH], F32, tag="ag_in")
            ag_out = dram.tile([256, SH], F32, tag="ag_out")
            nc.sync.dma_start(ag_in[:, :], kvt_sb[:, :])
            nc.gpsimd.collective_compute(
                "AllGather",
                mybir.AluOpType.bypass,
                replica_groups=[[0, 1], [2, 3], [4, 5], [6, 7]],
                ins=[ag_in.opt()],
                outs=[ag_out.opt()],
            )

            # Q projection overlaps the collective.
            _projection(nc, ctx, tc, xt_pool, psum_proj, qt_d, w_sb, 0,
                        b_sb["q"], lambda st: qt_sb[:, st * SB:(st + 1) * SB])

        # --- load gathered kT/vT ----------------------------------------
        kv_full = []
        for r in range(2):
            t = persist.tile([128, SH], F32R, tag=f"kvfull{r}")
            nc.sync.dma_start(t[:, :], ag_out.bitcast(F32R)[r * 128:(r + 1) * 128, :])
            kv_full.append(t)

        # --- v natural layout (bf16, with ones column) -------------------
        # v1[:, t*65:(t+1)*65] = [v_rows(t-tile) | 1]
        v1 = persist.tile([128, 2 * NTT * (DK + 1)], BF16, tag="v1")
        v1_3d = v1[:, :].rearrange("p (t n) -> p t n", n=DK + 1)
        nc.vector.memset(v1_3d[:, :, DK:DK + 1], 1.0)
        with tc.tile_pool(name="psum_tr", bufs=2, space="PSUM") as psum_tr:
            for r in range(2):
                for tt in range(NTT):
                    pt = psum_tr.tile([128, DK], F32R, tag="tr")
                    nc.tensor.transpose(
                        pt[:, :],
                        kv_full[r][DK:128, tt * 128:(tt + 1) * 128],
                        ident[DK:128, DK:128],
                    )
                    nc.scalar.activation(
                        v1_3d[:, r * NTT + tt, 0:DK], pt[:, :],
                        mybir.ActivationFunctionType.Copy,
                    )

        # --- attention ---------------------------------------------------
        pt_pool = ctx.enter_context(tc.tile_pool(name="pt", bufs=40))
        psum_st = ctx.enter_context(
            tc.tile_pool(name="psum_st", bufs=4, space="PSUM"))
        psum_pv = ctx.enter_context(
            tc.tile_pool(name="psum_pv", bufs=2, space="PSUM"))
        opool = ctx.enter_context(tc.tile_pool(name="opool", bufs=4))

        for sblk in range(NSB):
            q_ap = qt_sb[:, sblk * SB:(sblk + 1) * SB]
            pts = []
            for r in range(2):
                for tt in range(NTT):
                    ps = psum_st.tile([128, SB], F32, tag="st")
                    nc.tensor.matmul(
                        ps[:, :],
                        lhsT=kv_full[r][0:DK, tt * 128:(tt + 1) * 128],
                        rhs=q_ap,
                        start=True, stop=True,
                    )
                    pt = pt_pool.tile([128, SB], BF16, tag="pt")
                    nc.scalar.activation(
                        pt[:, :], ps[:, :],
                        mybir.ActivationFunctionType.Exp, scale=0.125,
                    )
                    pts.append(pt)
            for ss in range(SB // 128):
                po = psum_pv.tile([128, DK + 1], F32, tag="pv")
                for t in range(2 * NTT):
                    nc.tensor.matmul(
                        po[:, :],
                        lhsT=pts[t][:, ss * 128:(ss + 1) * 128],
                        rhs=v1_3d[:, t],
                        start=(t == 0), stop=(t == 2 * NTT - 1),
                    )
                rec = opool.tile([128, 1], F32, tag="rec")
                nc.vector.reciprocal(rec[:, :], po[:, DK:DK + 1])
                o_sb = opool.tile([128, DK], F32, tag="osb")
                nc.vector.tensor_scalar_mul(o_sb[:, :], po[:, 0:DK], rec[:, :])
                row0 = sblk * SB + ss * 128
                nc.sync.dma_start(out_d.ap()[row0:row0 + 128, :], o_sb[:, :])

    nc.compile()
    return nc


def kernel(Q, K, V, Wq, bq, Wk, bk, Wv, bv):
    global _compiled
    if _compiled is None:
        _compiled = _build()
    nc = _compiled

    Q = np.asarray(Q, dtype=np.float32)
    K = np.asarray(K, dtype=np.float32)
    V = np.asarray(V, dtype=np.float32)
    shared = {
        "Wq": np.ascontiguousarray(Wq, dtype=np.float32),
        "Wk": np.ascontiguousarray(Wk, dtype=np.float32),
        "Wv": np.ascontiguousarray(Wv, dtype=np.float32),
        "bq": np.ascontiguousarray(bq, dtype=np.float32).reshape(DK, 1),
        "bk": np.ascontiguousarray(bk, dtype=np.float32).reshape(DK, 1),
        "bv": np.ascontiguousarray(bv, dtype=np.float32).reshape(DK, 1),
    }
    in_maps = []
    for c in range(N_CORES):
        b, h = c // 2, c % 2
        sl = slice(h * SH, (h + 1) * SH)
        in_maps.append({
            "QT": np.ascontiguousarray(Q[b, sl, :].T),
            "KT": np.ascontiguousarray(K[b, sl, :].T),
            "VT": np.ascontiguousarray(V[b, sl, :].T),
            **shared,
        })

    res = run_bass_kernel_spmd(nc, in_maps, core_ids=list(range(N_CORES)))

    out = np.empty((B, S, DK), dtype=np.float32)
    for c in range(N_CORES):
        b, h = c // 2, c % 2
        out[b, h * SH:(h + 1) * SH, :] = res.results[c]["out"]
    return out
